# revision 1
# baseline (speedup 1.0000x reference)
"""DynamicConv1dTBC (T=2048, B=4, C=512, H=16, K=31, P=15) on 8 trn2 cores.

Sequence-parallel over T (8 x 256 rows; halo via host-side zero-padded
slabs).  Activations run in transposed layout [C, (t,b)] so all large
matmuls contract channels on partitions.  The dynamic conv itself runs on
the PE as per-(b,head,t-chunk) banded matmuls.  Band matrices are
materialized with a DRAM round trip: softmax weights are scattered into Z
rows shaped [81 zeros | 31 taps] (pitch 112) and read back with a
stride-111 window access pattern, which lands row p's taps at columns
[p+17, p+47] of a 128-wide window -- i.e. the band -- using only
rectangular SBUF access patterns.
"""

import numpy as np
import ml_dtypes

import sys
if "/opt/trn_rl_repo" not in sys.path:
    sys.path.insert(0, "/opt/trn_rl_repo")

import concourse.bass as bass
import concourse.mybir as mybir

T, B, C = 2048, 4, 512
H, K, P = 16, 31, 15
R = C // H
NCORES = 8
TLOC = T // NCORES            # 256
PAD = 32
SLAB_T = TLOC + 2 * PAD       # 320
SLAB = SLAB_T * B             # 1280
OWN = TLOC * B                # 1024
HK = H * K                    # 496
TC = 64
NTC = TLOC // TC              # 4
WINC = 128                    # conv contraction window (t' rows per chunk)
ZPITCH = 112                  # Z row pitch: 81 zeros + 31 taps
ZLEAD = ZPITCH - K            # 81
ZH = TLOC * ZPITCH + 32       # 28704 (incl. tail pad)
ZB = H * ZH
ZTOT = B * ZB
BD_W = H * WINC               # 2048  (per (b,tc) band tile width)
BDT_W = H * TC                # 1024

NXT = SLAB // 128             # 10 x-tiles
GN = (512, 512, 256)          # lin1 tb groups
JN = (128, 128, 64)           # slab s-blocks for h re-transpose

f32 = mybir.dt.float32
bf16 = mybir.dt.bfloat16
ENGS = ("sync", "vector", "scalar", "gpsimd", "tensor")


def _ap(t, offset, dims):
    return bass.AP(tensor=t.tensor if hasattr(t, "tensor") else t,
                   offset=offset, ap=[list(d) for d in dims])


def build_nc():
    nc = bass.Bass()

    xslab = nc.declare_dram_parameter("xslab", [SLAB, C], f32, isOutput=False)
    w1t = nc.declare_dram_parameter("w1t", [C, C], bf16, isOutput=False)
    wlt = nc.declare_dram_parameter("wlt", [C, HK], bf16, isOutput=False)
    w2t = nc.declare_dram_parameter("w2t", [C, C], bf16, isOutput=False)
    b1p = nc.declare_dram_parameter("b1p", [128, 4], f32, isOutput=False)
    b2p = nc.declare_dram_parameter("b2p", [128, 4], f32, isOutput=False)
    idp = nc.declare_dram_parameter("idp", [128, 128], bf16, isOutput=False)
    out = nc.declare_dram_parameter("out", [OWN, C], f32, isOutput=True)
    zdr = nc.dram_tensor("zscratch", [ZTOT], bf16)

    steps = {e: [] for e in ENGS}
    cnt = {}

    def step(eng, emit, waits=(), inc=None):
        mil = None
        if inc is not None:
            sem, amt = inc
            cnt[id(sem)] = cnt.get(id(sem), 0) + amt
            mil = (sem, cnt[id(sem)])
        steps[eng].append((emit, tuple(w for w in waits if w), inc))
        return mil

    slots = {}

    from contextlib import ExitStack
    with ExitStack() as ctx:
        en = ctx.enter_context
        zt = en(nc.sbuf_tensor([128, 1024], bf16))
        ident = en(nc.sbuf_tensor([128, 128], bf16))
        xstg = en(nc.sbuf_tensor([128, 2 * C], f32))
        xnb = en(nc.sbuf_tensor([128, NXT * C], bf16))
        w1s = en(nc.sbuf_tensor([128, 2048], bf16))
        wls = en(nc.sbuf_tensor([128, 4 * HK], bf16))
        w2s = en(nc.sbuf_tensor([128, 2048], bf16))
        b1s = en(nc.sbuf_tensor([128, 4], f32))
        b2s = en(nc.sbuf_tensor([128, 4], f32))
        xts = en(nc.sbuf_tensor([128, 4 * SLAB], bf16))
        hts = en(nc.sbuf_tensor([128, 4 * SLAB], bf16))
        hbw = en(nc.sbuf_tensor([128, 16 * C], bf16))
        ews = en(nc.sbuf_tensor([128, 2 * HK], f32))
        wns = en(nc.sbuf_tensor([128, 8 * HK], bf16))
        zss = en(nc.sbuf_tensor([128, 2 * H], f32))
        rzs = en(nc.sbuf_tensor([128, 2 * H], f32))
        bnd = en(nc.sbuf_tensor([64, 16 * BD_W], bf16))
        bndt = en(nc.sbuf_tensor([128, 16 * BDT_W], bf16))
        cvt = en(nc.sbuf_tensor([128, 4 * OWN], bf16))
        o2t = en(nc.sbuf_tensor([128, 4 * OWN], bf16))
        onat = en(nc.sbuf_tensor([128, 8 * C], f32))
        ph0 = en(nc.psum_tensor([128, 512], f32))
        ph1 = en(nc.psum_tensor([128, 512], f32))
        pt0 = en(nc.psum_tensor([128, 512], bf16))
        pt1 = en(nc.psum_tensor([128, 512], bf16))
        pw0 = en(nc.psum_tensor([128, 512], bf16))
        pw1 = en(nc.psum_tensor([128, 512], bf16))
        pc0 = en(nc.psum_tensor([64, 512], f32))
        pc1 = en(nc.psum_tensor([64, 512], f32))
        s_wl = en(nc.semaphore("s_wl"))
        s_x0 = en(nc.semaphore("s_x0"))
        s_x1 = en(nc.semaphore("s_x1"))
        s_zz = en(nc.semaphore("s_zz"))
        s_zw = en(nc.semaphore("s_zw"))
        s_br = en(nc.semaphore("s_br"))
        s_pe = en(nc.semaphore("s_pe"))
        s_act = en(nc.semaphore("s_act"))
        s_dve = en(nc.semaphore("s_dve"))
        s_out = en(nc.semaphore("s_out"))
        block = en(nc.Block())
        ph = [ph0, ph1]
        pt = [pt0, pt1]
        pw = [pw0, pw1, pt1]           # 3rd h-block borrows pt1
        pwkey = (("pw", 0), ("pw", 1), ("pt", 1))
        pc = [pc0, pc1]

        # ---------------- loads, memset, z-zero ----------------
        def ld(dst_ap, src_ap):
            return step("sync", lambda e, d=dst_ap, s=src_ap: e.dma_start(d, s),
                        inc=(s_wl, 16))

        for cc in range(4):
            ld(w1s[:, 512 * cc:512 * cc + 512],
               _ap(w1t, 128 * cc * C, [[C, 128], [1, C]]))
            ld(wls[:, HK * cc:HK * cc + HK],
               _ap(wlt, 128 * cc * HK, [[HK, 128], [1, HK]]))
            ld(w2s[:, 512 * cc:512 * cc + 512],
               _ap(w2t, 128 * cc * C, [[C, 128], [1, C]]))
        ld(b1s[:], b1p[:])
        ld(b2s[:], b2p[:])
        m_wld = ld(ident[:], idp[:])

        m_zt = step("vector", lambda e: nc.vector.memset(zt[:], 0.0),
                    inc=(s_dve, 1))
        nzz = ZTOT // 65536            # full 65536-elem chunks
        zrem = ZTOT - nzz * 65536
        for j in range(nzz + (1 if zrem else 0)):
            rows = 128 if j < nzz else zrem // 512
            step("sync",
                 lambda e, j=j, rows=rows: e.dma_start(
                     _ap(zdr, 65536 * j, [[512, rows], [1, 512]]),
                     zt[:rows, :512]),
                 waits=[m_zt] if j == 0 else (), inc=(s_zz, 16))
        m_zzero = (s_zz, cnt[id(s_zz)])

        # ---------------- x load + cast + transpose ----------------
        m_cast = {}
        m_xtc = {}
        for i in range(NXT):
            w = [m_cast[i - 2]] if i >= 2 else []
            sx = (s_x0, s_x1)[i % 2]
            m_x = step("sync",
                       lambda e, i=i: e.dma_start(
                           xstg[:, 512 * (i % 2):512 * (i % 2) + 512],
                           _ap(xslab, 128 * i * C, [[C, 128], [1, C]])),
                       waits=w, inc=(sx, 16))
            m_cast[i] = step(
                "vector",
                lambda e, i=i: nc.vector.tensor_copy(
                    xnb[:, 512 * i:512 * i + 512],
                    xstg[:, 512 * (i % 2):512 * (i % 2) + 512]),
                waits=[m_x], inc=(s_dve, 1))

        for i in range(NXT):
            wts = [m_wld, m_cast[i], slots.get(("pt", i % 2))]

            def tr_x(e, i=i):
                last = None
                for cc in range(4):
                    last = nc.tensor.transpose(
                        pt[i % 2][:, 128 * cc:128 * cc + 128],
                        xnb[:, 512 * i + 128 * cc:512 * i + 128 * cc + 128],
                        ident[:])
                return last
            m = step("tensor", tr_x, waits=wts, inc=(s_pe, 1))
            m_xtc[i] = step(
                "vector",
                lambda e, i=i: nc.vector.tensor_copy(
                    xts[:, 512 * i:512 * i + 512], pt[i % 2][:, :512]),
                waits=[m], inc=(s_dve, 1))
            slots[("pt", i % 2)] = m_xtc[i]

        # ---------------- linear1 -> hT ----------------
        m_h = {}
        gdep = (m_xtc[3], m_xtc[7], m_xtc[9])
        for dc in range(4):
            for g in range(3):
                it = dc * 3 + g
                wts = [gdep[g], slots.get(("ph", it % 2))]

                def mm1(e, dc=dc, g=g, it=it):
                    ncols = GN[g]
                    ntile = ncols // 128
                    last = None
                    for cc in range(4):
                        rhs = _ap(xts, 512 * 4 * g + 128 * cc,
                                  [[4 * SLAB, 128], [512, ntile], [1, 128]])
                        last = nc.tensor.matmul(
                            ph[it % 2][:, :ncols],
                            w1s[:, 512 * cc + 128 * dc:512 * cc + 128 * dc + 128],
                            rhs, start=(cc == 0), stop=(cc == 3))
                    return last
                m = step("tensor", mm1, waits=wts, inc=(s_pe, 1))
                m_h[it] = step(
                    "scalar",
                    lambda e, dc=dc, g=g, it=it: nc.scalar.activation(
                        hts[:, SLAB * dc + 512 * g:SLAB * dc + 512 * g + GN[g]],
                        ph[it % 2][:, :GN[g]],
                        mybir.ActivationFunctionType.Identity,
                        bias=b1s[:, dc:dc + 1], scale=1.0),
                    waits=[m], inc=(s_act, 1))
                slots[("ph", it % 2)] = m_h[it]

        # ---------------- h re-transpose -> hbw window tiles ----------------
        # window for (b,tc): slab s rows [64*tc, 64*tc+128)
        m_htr = {}
        m_hbw_all = {}
        for b in range(4):
            for j in range(3):
                r0_, r1_ = SLAB_T * b + 128 * j, SLAB_T * b + 128 * j + JN[j]
                gneed = [g for g in range(3)
                         if r0_ < 512 * g + GN[g] and 512 * g < r1_]
                wts = [m_h[dc * 3 + g] for dc in range(4) for g in gneed]
                wts.append(slots.get(pwkey[j]))

                def tr_h(e, b=b, j=j):
                    last = None
                    for dc in range(4):
                        in_ = _ap(hts, SLAB * dc + SLAB_T * b + 128 * j,
                                  [[4 * SLAB, 128], [1, JN[j]]])
                        last = nc.tensor.transpose(
                            pw[j][:JN[j], 128 * dc:128 * dc + 128],
                            in_, ident[:])
                    return last
                m_htr[(b, j)] = step("tensor", tr_h, waits=wts, inc=(s_pe, 1))

            m_hbw = {}
            for tcc in range(4):
                idx = 4 * b + tcc
                lo = 64 * tcc               # aligned window start (slab s)
                j0, r0 = divmod(lo, 128)    # r0 in {0, 64}
                take0 = min(WINC, 128 - r0)
                wts = [m_htr[(b, j0)]]
                if take0 < WINC:
                    wts.append(m_htr[(b, j0 + 1)])

                def cp_hbw(e, idx=idx, j0=j0, r0=r0, take0=take0):
                    last = nc.scalar.copy(
                        hbw[0:take0, 512 * idx:512 * idx + 512],
                        pw[j0][r0:r0 + take0, :512])
                    if take0 < WINC:
                        last = nc.scalar.copy(
                            hbw[take0:WINC, 512 * idx:512 * idx + 512],
                            pw[j0 + 1][0:WINC - take0, :512])
                    return last
                m_hbw[idx] = step("scalar", cp_hbw, waits=wts, inc=(s_act, 1))
            slots[pwkey[0]] = m_hbw[4 * b + 1]
            slots[pwkey[1]] = m_hbw[4 * b + 3]
            slots[pwkey[2]] = m_hbw[4 * b + 3]
            m_hbw_all.update(m_hbw)

        # ---------------- wlog + softmax + z-write ----------------
        m_nm = {}
        m_zw = {}
        for i in range(8):
            wb, half = i // 2, i % 2
            ocol = SLAB_T * wb + PAD + 128 * half   # owned tile start (b-major)
            tb0, tb1 = ocol, ocol + 128
            gset = [g for g in range(3)
                    if tb0 < 512 * g + GN[g] and 512 * g < tb1]
            wts = [m_h[dc * 3 + g] for dc in range(4) for g in gset]
            wts.append(slots.get(("ph", i % 2)))

            def mmw(e, i=i, ocol=ocol):
                last = None
                for cc in range(4):
                    lhsT = _ap(hts, SLAB * cc + ocol,
                               [[4 * SLAB, 128], [1, 128]])
                    last = nc.tensor.matmul(
                        ph[i % 2][:, :HK], lhsT,
                        wls[:, HK * cc:HK * cc + HK],
                        start=(cc == 0), stop=(cc == 3))
                return last
            m_wl = step("tensor", mmw, waits=wts, inc=(s_pe, 1))

            wts = [m_wl]
            if i >= 2:
                wts.append(m_nm[i - 2])
            m_ex = step(
                "scalar",
                lambda e, i=i: nc.scalar.activation(
                    ews[:, (i % 2) * HK:(i % 2) * HK + HK],
                    ph[i % 2][:, :HK],
                    mybir.ActivationFunctionType.Exp),
                waits=wts, inc=(s_act, 1))
            slots[("ph", i % 2)] = m_ex

            m_rd = step(
                "vector",
                lambda e, i=i: nc.vector.tensor_reduce(
                    zss[:, (i % 2) * H:(i % 2) * H + H],
                    _ap(ews, (i % 2) * HK, [[2 * HK, 128], [K, H], [1, K]]),
                    op=mybir.AluOpType.add, axis=mybir.AxisListType.X),
                waits=[m_ex], inc=(s_dve, 1))
            m_rc = step(
                "vector",
                lambda e, i=i: nc.vector.reciprocal(
                    rzs[:, (i % 2) * H:(i % 2) * H + H],
                    zss[:, (i % 2) * H:(i % 2) * H + H]),
                waits=[m_rd], inc=(s_dve, 1))
            m_nm[i] = step(
                "vector",
                lambda e, i=i: nc.vector.tensor_tensor(
                    out=_ap(wns, i * HK, [[8 * HK, 128], [K, H], [1, K]]),
                    in0=_ap(ews, (i % 2) * HK, [[2 * HK, 128], [K, H], [1, K]]),
                    in1=_ap(rzs, (i % 2) * H, [[2 * H, 128], [1, H], [0, K]]),
                    op=mybir.AluOpType.mult),
                waits=[m_rc], inc=(s_dve, 1))

            wts = [m_nm[i]]
            if i == 0:
                wts.append(m_zzero)

            def zw(e, i=i, wb=wb, half=half):
                src = _ap(wns, i * HK,
                          [[8 * HK, 128], [K, H], [1, K]])
                dst = _ap(zdr, wb * ZB + (128 * half) * ZPITCH + ZLEAD,
                          [[ZPITCH, 128], [ZH, H], [1, K]])
                return e.dma_start(dst, src)
            m_zw[i] = step("sync", zw, waits=wts, inc=(s_zw, 16))

        # ---------------- band read + transpose + conv ----------------
        # band row p (t = 64*tc+p) gets taps at cols [p+17, p+47]:
        #   window start addr = (64*tc+p)*112 + 81 - (p+17) = 7168*tc + 64 + 111*p
        m_cv = {}
        m_zw_all = (s_zw, cnt[id(s_zw)])
        for b in range(4):
            for tcc in range(4):
                idx = 4 * b + tcc

                def br(e, b=b, tcc=tcc, idx=idx):
                    src = _ap(zdr, b * ZB + 7168 * tcc + 64,
                              [[ZPITCH - 1, 64], [ZH, H], [1, WINC]])
                    dst = _ap(bnd, BD_W * idx,
                              [[16 * BD_W, 64], [WINC, H], [1, WINC]])
                    return e.dma_start(dst, src)
                step("sync", br, waits=[m_zw_all], inc=(s_br, 16))
        m_br_all = (s_br, cnt[id(s_br)])

        for b in range(4):
            for tcc in range(4):
                idx = 4 * b + tcc
                m_half = []
                for half in range(2):
                    wts = [m_br_all, slots.get(("pt", half))]

                    def tr_b(e, idx=idx, half=half):
                        last = None
                        for hh in range(8):
                            h = 8 * half + hh
                            in_ = _ap(bnd, BD_W * idx + WINC * h,
                                      [[16 * BD_W, 64], [1, WINC]])
                            last = nc.tensor.transpose(
                                pt[half][:WINC, 64 * hh:64 * hh + 64],
                                in_, ident[:64, :64])
                        return last
                    m_tb = step("tensor", tr_b, waits=wts, inc=(s_pe, 1))
                    m_cp = step(
                        "vector",
                        lambda e, idx=idx, half=half: nc.vector.tensor_copy(
                            bndt[:, BDT_W * idx + 512 * half:
                                 BDT_W * idx + 512 * half + 512],
                            pt[half][:, :512]),
                        waits=[m_tb], inc=(s_dve, 1))
                    slots[("pt", half)] = m_cp
                    m_half.append(m_cp)

                wts = m_half + [m_hbw_all[idx], slots.get(("pc", idx % 2))]

                def cv(e, idx=idx):
                    last = None
                    for h in range(H):
                        lhsT = hbw[:, 512 * idx + 32 * h:512 * idx + 32 * h + 32]
                        rhs = bndt[:, BDT_W * idx + 64 * h:BDT_W * idx + 64 * h + 64]
                        last = nc.tensor.matmul(
                            pc[idx % 2][32 * (h % 2):32 * (h % 2) + 32,
                                        64 * (h // 2):64 * (h // 2) + 64],
                            lhsT, rhs, start=True, stop=True)
                    return last
                m_c = step("tensor", cv, waits=wts, inc=(s_pe, 1))

                def cvcp(e, idx=idx, b=b, tcc=tcc):
                    # psum col-group cg holds heads (2cg, 2cg+1); cvt chunk
                    # cg//2 at partition offset 64*(cg%2)
                    last = None
                    for cg in range(8):
                        cc = cg // 2
                        poff = 64 * (cg % 2)
                        last = nc.vector.tensor_copy(
                            _ap(cvt, poff * (4 * OWN) + OWN * cc + 256 * b + 64 * tcc,
                                [[4 * OWN, 64], [1, TC]]),
                            pc[idx % 2][0:64, 64 * cg:64 * cg + 64])
                    return last
                m_cv[idx] = step("vector", cvcp, waits=[m_c], inc=(s_dve, 1))
                slots[("pc", idx % 2)] = m_cv[idx]

        # ---------------- linear2 -> out2T ----------------
        m_o2 = {}
        for dc in range(4):
            for g2 in range(2):
                it = dc * 2 + g2
                wts = [m_cv[15], slots.get(("ph", it % 2))]

                def mm2(e, dc=dc, g2=g2, it=it):
                    last = None
                    for cc in range(4):
                        last = nc.tensor.matmul(
                            ph[it % 2][:, :512],
                            w2s[:, 512 * cc + 128 * dc:512 * cc + 128 * dc + 128],
                            cvt[:, OWN * cc + 512 * g2:OWN * cc + 512 * g2 + 512],
                            start=(cc == 0), stop=(cc == 3))
                    return last
                m = step("tensor", mm2, waits=wts, inc=(s_pe, 1))
                m_o2[it] = step(
                    "scalar",
                    lambda e, dc=dc, g2=g2, it=it: nc.scalar.activation(
                        o2t[:, OWN * dc + 512 * g2:OWN * dc + 512 * g2 + 512],
                        ph[it % 2][:, :512],
                        mybir.ActivationFunctionType.Identity,
                        bias=b2s[:, dc:dc + 1], scale=1.0),
                    waits=[m], inc=(s_act, 1))
                slots[("ph", it % 2)] = m_o2[it]

        # ---------------- transpose back + store ----------------
        for i in range(8):
            g2 = i // 4
            wts = [m_o2[dc * 2 + g2] for dc in range(4)]
            wts.append(slots.get(("pw", i % 2)))

            def tr_o(e, i=i):
                last = None
                for dc in range(4):
                    last = nc.tensor.transpose(
                        pw[i % 2][:, 128 * dc:128 * dc + 128],
                        o2t[:, OWN * dc + 128 * i:OWN * dc + 128 * i + 128],
                        ident[:])
                return last
            m = step("tensor", tr_o, waits=wts, inc=(s_pe, 1))
            m_on = step(
                "vector",
                lambda e, i=i: nc.vector.tensor_copy(
                    onat[:, 512 * i:512 * i + 512], pw[i % 2][:, :512]),
                waits=[m], inc=(s_dve, 1))
            slots[("pw", i % 2)] = m_on
            step("sync",
                 lambda e, i=i: e.dma_start(
                     _ap(out, 128 * i * C, [[C, 128], [1, C]]),
                     onat[:, 512 * i:512 * i + 512]),
                 waits=[m_on], inc=(s_out, 16))

        # ---------------- run per-engine step lists ----------------
        def runner(eng_obj, name):
            for emit, waits, inc in steps[name]:
                for sem, val in waits:
                    eng_obj.wait_ge(sem, val)
                inst = emit(eng_obj)
                if inc is not None:
                    sem, amt = inc
                    if inst is None:
                        inst = eng_obj.engine_nop()
                    inst.then_inc(sem, amt)

        @block.sync
        def _(eng):
            runner(eng, "sync")

        @block.vector
        def _(eng):
            runner(eng, "vector")

        @block.scalar
        def _(eng):
            runner(eng, "scalar")

        @block.gpsimd
        def _(eng):
            runner(eng, "gpsimd")

        @block.tensor
        def _(eng):
            runner(eng, "tensor")

    return nc


_NC = None


def _get_nc():
    global _NC
    if _NC is None:
        _NC = build_nc()
    return _NC


def _prep_inputs(x, W1, b1, Wl, W2, b2):
    xf = np.asarray(x, np.float32)
    xp = np.zeros((T + 2 * PAD, B, C), np.float32)
    xp[PAD:PAD + T] = xf
    w1t = np.ascontiguousarray(np.asarray(W1, np.float32).T).astype(ml_dtypes.bfloat16)
    wlt = np.ascontiguousarray(np.asarray(Wl, np.float32).T).astype(ml_dtypes.bfloat16)
    w2t = np.ascontiguousarray(np.asarray(W2, np.float32).T).astype(ml_dtypes.bfloat16)
    b1p = np.ascontiguousarray(np.asarray(b1, np.float32).reshape(4, 128).T)
    b2p = np.ascontiguousarray(np.asarray(b2, np.float32).reshape(4, 128).T)
    idp = np.eye(128, dtype=np.float32).astype(ml_dtypes.bfloat16)
    maps = []
    for c in range(NCORES):
        slab = np.ascontiguousarray(
            xp[c * TLOC:c * TLOC + SLAB_T].transpose(1, 0, 2).reshape(SLAB, C))
        maps.append({"xslab": slab, "w1t": w1t, "wlt": wlt, "w2t": w2t,
                     "b1p": b1p, "b2p": b2p, "idp": idp})
    return maps


def kernel(x, W1, b1, Wl, W2, b2):
    from concourse.bass_utils import run_bass_kernel_spmd
    nc = _get_nc()
    maps = _prep_inputs(x, W1, b1, Wl, W2, b2)
    res = run_bass_kernel_spmd(nc, maps, list(range(NCORES)))
    outs = [res.results[c]["out"].reshape(B, TLOC, C).transpose(1, 0, 2)
            for c in range(NCORES)]
    return np.concatenate(outs, axis=0)



# revision 18
# speedup vs baseline: 1.7214x; 1.7214x over previous
"""DynamicConv1dTBC (T=2048, B=4, C=512, H=16, K=31, P=15) on 8 trn2 cores.

Sequence-parallel over T (8 x 256 rows; halo via host-side zero-padded
slabs).  Host pre-transposes x to bf16 [C, SLAB] and un-transposes the
f32 [C, OWN] output, so the PE only does the h re-transpose and the
band transposes.  The dynamic conv runs as per-(b,head,t-chunk) banded
matmuls; band matrices are materialized via a DRAM round trip with
contiguous row writes: softmax weights land in z rows [t, h*112] where
each 112-block is [81 zeros | 31 taps] (zeros come from a pre-zeroed
SBUF staging buffer), and a stride-1791 window read lands row p's taps
at columns [p+17, p+47] of the 128-wide window.  Only the 96-column
support [16,112) is read back.  The whole kernel is software-pipelined
over the batch dim b: lin1(b)/wlog(b)/softmax(b) -> z round trip(b)
overlaps PE work of b+1; z writes issue on the scalar (Act) DMA queue,
band reads on the sync (SP) queue.
"""

import numpy as np
import ml_dtypes

import sys
if "/opt/trn_rl_repo" not in sys.path:
    sys.path.insert(0, "/opt/trn_rl_repo")

import concourse.bass as bass
import concourse.mybir as mybir

T, B, C = 2048, 4, 512
H, K, P = 16, 31, 15
R = C // H
NCORES = 8
TLOC = T // NCORES            # 256
PAD = 32
SLAB_T = TLOC + 2 * PAD       # 320
SLAB = SLAB_T * B             # 1280
OWN = TLOC * B                # 1024
HK = H * K                    # 496
TC = 64
WINC = 128                    # full conv window (t' rows per chunk)
WSUP = 112                    # band support rows read back: window [0,112)
WLO = 0
ZROW = H * 112                # 1792: z row = per-t [16 x (81 zeros | 31 taps)]
ZB6 = (TLOC + 1) * ZROW       # per-b z block incl. zero pad row
ZTOT = B * ZB6
BD_W = H * WSUP               # 1536 (per (b,tc) band tile width, as-read)
BDT_W = H * TC                # 1024 (transposed band tile width)
JN = (128, 128, 64)           # slab s-blocks for h re-transpose

f32 = mybir.dt.float32
bf16 = mybir.dt.bfloat16
ENGS = ("sync", "vector", "scalar", "gpsimd", "tensor")


def _ap(t, offset, dims):
    return bass.AP(tensor=t.tensor if hasattr(t, "tensor") else t,
                   offset=offset, ap=[list(d) for d in dims])


def build_nc():
    nc = bass.Bass()

    xslabt = nc.declare_dram_parameter("xslabt", [C, SLAB], bf16, isOutput=False)
    w1t = nc.declare_dram_parameter("w1t", [C, C], bf16, isOutput=False)
    wlt = nc.declare_dram_parameter("wlt", [C, HK], bf16, isOutput=False)
    w2t = nc.declare_dram_parameter("w2t", [C, C], bf16, isOutput=False)
    b1p = nc.declare_dram_parameter("b1p", [128, 4], f32, isOutput=False)
    b2p = nc.declare_dram_parameter("b2p", [128, 4], f32, isOutput=False)
    idp = nc.declare_dram_parameter("idp", [128, 128], bf16, isOutput=False)
    out = nc.declare_dram_parameter("out", [C, OWN], f32, isOutput=True)
    zdr = nc.dram_tensor("zscratch", [ZTOT], bf16)

    steps = {e: [] for e in ENGS}
    cnt = {}

    def step(eng, emit, waits=(), inc=None):
        mil = None
        if inc is not None:
            sem, amt = inc
            cnt[id(sem)] = cnt.get(id(sem), 0) + amt
            mil = (sem, cnt[id(sem)])
        steps[eng].append((emit, tuple(w for w in waits if w), inc))
        return mil

    slots = {}

    from contextlib import ExitStack
    with ExitStack() as ctx:
        en = ctx.enter_context
        ident = en(nc.sbuf_tensor([128, 128], bf16))
        xts = en(nc.sbuf_tensor([128, 4 * SLAB], bf16))
        w1s = en(nc.sbuf_tensor([128, 2048], bf16))
        wls = en(nc.sbuf_tensor([128, 4 * HK], bf16))
        w2s = en(nc.sbuf_tensor([128, 2048], bf16))
        b1s = en(nc.sbuf_tensor([128, 4], f32))
        b2s = en(nc.sbuf_tensor([128, 4], f32))
        hts = en(nc.sbuf_tensor([128, 4 * SLAB], bf16))
        hbw = en(nc.sbuf_tensor([128, 16 * C], bf16))
        ews = en(nc.sbuf_tensor([128, 2 * HK], f32))
        stg = en(nc.sbuf_tensor([128, 3 * ZROW], bf16))
        zss = en(nc.sbuf_tensor([128, 2 * H], f32))
        rzs = en(nc.sbuf_tensor([128, 2 * H], f32))
        bnd = en(nc.sbuf_tensor([64, 16 * BD_W], bf16))
        bndt = en(nc.sbuf_tensor([128, 16 * BDT_W], bf16))
        cvt = en(nc.sbuf_tensor([128, 4 * OWN], bf16))
        o2t = en(nc.sbuf_tensor([128, 4 * OWN], f32))
        ph0 = en(nc.psum_tensor([128, 512], f32))
        ph1 = en(nc.psum_tensor([128, 512], f32))
        pwb = en(nc.psum_tensor([128, 1024], bf16))  # halves: pw0 | pw1
        pw2 = en(nc.psum_tensor([128, 512], bf16))
        pt0 = en(nc.psum_tensor([128, 512], bf16))
        pt1 = en(nc.psum_tensor([128, 512], bf16))
        pc0 = en(nc.psum_tensor([64, 512], f32))
        pc1 = en(nc.psum_tensor([64, 512], f32))
        s_xw = en(nc.semaphore("s_xw"))          # x0 + w1
        s_wl = en(nc.semaphore("s_wl"))
        s_xb = [en(nc.semaphore(f"s_x{b}")) for b in range(1, 4)]
        s_misc = en(nc.semaphore("s_misc"))      # ident + b1 + b2
        s_w2 = en(nc.semaphore("s_w2"))
        s_pad = en(nc.semaphore("s_pad"))
        s_zw2 = [[en(nc.semaphore(f"s_zw{b}{h}")) for h in range(2)]
                 for b in range(4)]
        s_bri = [en(nc.semaphore(f"s_br{i}")) for i in range(16)]
        s_pe = en(nc.semaphore("s_pe"))
        s_act = en(nc.semaphore("s_act"))
        s_dve = en(nc.semaphore("s_dve"))
        s_out = en(nc.semaphore("s_out"))
        block = en(nc.Block())
        ph = [ph0, ph1]
        pt = [pt0, pt1]
        pc = [pc0, pc1]

        def pw_ap(j, rows, cols):
            # pw0/pw1 live in pwb halves; pw2 is its own bank
            r0, r1 = rows
            c0, c1 = cols
            if j < 2:
                return pwb[r0:r1, 512 * j + c0:512 * j + c1]
            return pw2[r0:r1, c0:c1]

        # ---------------- loads (sync queue) ----------------
        # each wait-point has its own semaphore; waits are on group totals
        # so concurrent DMA completion order never matters.
        def ld(sem, dst_ap, src_ap, waits=()):
            return step("sync", lambda e, d=dst_ap, s=src_ap: e.dma_start(d, s),
                        waits=waits, inc=(sem, 16))

        m_x = {}
        ld(s_xw, _ap(xts, SLAB_T * 0, [[4 * SLAB, 128], [SLAB, 4], [1, SLAB_T]]),
           _ap(xslabt, SLAB_T * 0, [[SLAB, 128], [128 * SLAB, 4], [1, SLAB_T]]))
        m_x[0] = ld(s_xw, _ap(w1s, 0, [[2048, 128], [512, 4], [1, 512]]),
                    _ap(w1t, 0, [[C, 128], [128 * C, 4], [1, C]]))  # (s_xw, 32)
        m_w1 = m_x[0]
        m_wl = ld(s_wl, _ap(wls, 0, [[4 * HK, 128], [HK, 4], [1, HK]]),
                  _ap(wlt, 0, [[HK, 128], [128 * HK, 4], [1, HK]]))
        for b in range(1, 4):
            m_x[b] = ld(s_xb[b - 1],
                        _ap(xts, SLAB_T * b, [[4 * SLAB, 128], [SLAB, 4], [1, SLAB_T]]),
                        _ap(xslabt, SLAB_T * b, [[SLAB, 128], [128 * SLAB, 4], [1, SLAB_T]]))
        ld(s_misc, ident[:], idp[:])
        ld(s_misc, b1s[:], b1p[:])
        m_misc = ld(s_misc, b2s[:], b2p[:])   # (s_misc, 48) = all three
        m_id = m_b1 = m_b2 = m_misc
        m_w2 = ld(s_w2, _ap(w2s, 0, [[2048, 128], [512, 4], [1, 512]]),
                  _ap(w2t, 0, [[C, 128], [128 * C, 4], [1, C]]))

        # ---------------- stg memset + z pad row (vector + scalar) -------
        m_zt = step("vector", lambda e: nc.vector.memset(stg[:], 0.0),
                    inc=(s_dve, 1))
        m_pad = step("scalar",
                     lambda e: e.dma_start(
                         _ap(zdr, TLOC * ZROW, [[ZB6, 4], [1, ZROW]]),
                         _ap(stg, 2 * ZROW, [[3 * ZROW, 4], [1, ZROW]])),
                     waits=[m_zt], inc=(s_pad, 16))

        # ---------------- per-engine emit helpers ----------------
        phctr = [0]

        def next_ph():
            i = phctr[0] % 2
            phctr[0] += 1
            return i

        m_act = {}      # (b, dc) -> lin1 act milestone
        m_exp = {}      # (b, half) -> exp milestone
        m_nm = {}       # (b, half) -> normalize milestone
        m_zwm = {}      # (b, half) -> zwrite milestone
        m_brm = {}      # idx -> band read milestone
        m_cphbw = {}    # (b, tc) -> hbw copy milestone
        m_cpbnd = {}    # (idx, half) -> band transp copy milestone
        m_cvcp = {}     # idx -> conv psum copy milestone

        def emit_mm1(b):
            for dc in range(4):
                pi = next_ph()
                wts = [m_x[b], m_w1, slots.get(("ph", pi))]

                def mm1(e, b=b, dc=dc, pi=pi):
                    last = None
                    for cc in range(4):
                        last = nc.tensor.matmul(
                            ph[pi][:, :SLAB_T],
                            w1s[:, 512 * cc + 128 * dc:512 * cc + 128 * dc + 128],
                            _ap(xts, SLAB * cc + SLAB_T * b,
                                [[4 * SLAB, 128], [1, SLAB_T]]),
                            start=(cc == 0), stop=(cc == 3))
                    return last
                m = step("tensor", mm1, waits=wts, inc=(s_pe, 1))
                m_act[(b, dc)] = step(
                    "scalar",
                    lambda e, b=b, dc=dc, pi=pi: nc.scalar.activation(
                        hts[:, SLAB * dc + SLAB_T * b:
                            SLAB * dc + SLAB_T * b + SLAB_T],
                        ph[pi][:, :SLAB_T],
                        mybir.ActivationFunctionType.Identity,
                        bias=b1s[:, dc:dc + 1], scale=1.0),
                    waits=[m, m_b1], inc=(s_act, 1))
                slots[("ph", pi)] = m_act[(b, dc)]

        def emit_mmw(b, half):
            pi = next_ph()
            ocol = SLAB_T * b + PAD + 128 * half
            wts = [m_act[(b, dc)] for dc in range(4)]
            wts += [m_wl, slots.get(("ph", pi))]

            def mmw(e, b=b, half=half, pi=pi, ocol=ocol):
                last = None
                for cc in range(4):
                    last = nc.tensor.matmul(
                        ph[pi][:, :HK],
                        _ap(hts, SLAB * cc + ocol, [[4 * SLAB, 128], [1, 128]]),
                        wls[:, HK * cc:HK * cc + HK],
                        start=(cc == 0), stop=(cc == 3))
                return last
            m = step("tensor", mmw, waits=wts, inc=(s_pe, 1))
            i = 2 * b + half
            m_exp[(b, half)] = step(
                "scalar",
                lambda e, i=i, pi=pi: nc.scalar.activation(
                    ews[:, (i % 2) * HK:(i % 2) * HK + HK],
                    ph[pi][:, :HK],
                    mybir.ActivationFunctionType.Exp),
                waits=[m], inc=(s_act, 1))
            slots[("ph", pi)] = m_exp[(b, half)]

        def emit_softmax(b, half):
            i = 2 * b + half
            m_rd = step(
                "vector",
                lambda e, i=i: nc.vector.tensor_reduce(
                    zss[:, (i % 2) * H:(i % 2) * H + H],
                    _ap(ews, (i % 2) * HK, [[2 * HK, 128], [K, H], [1, K]]),
                    op=mybir.AluOpType.add, axis=mybir.AxisListType.X),
                waits=[m_exp[(b, half)]], inc=(s_dve, 1))
            m_rc = step(
                "vector",
                lambda e, i=i: nc.vector.reciprocal(
                    rzs[:, (i % 2) * H:(i % 2) * H + H],
                    zss[:, (i % 2) * H:(i % 2) * H + H]),
                waits=[m_rd], inc=(s_dve, 1))
            # write normalized taps into pre-zeroed staging block (i%2)
            wts = [m_rc, m_zt]
            if i >= 2:
                wts.append(m_zwm[((i - 2) // 2, (i - 2) % 2)])
            m_nm[(b, half)] = step(
                "vector",
                lambda e, i=i: nc.vector.tensor_tensor(
                    out=_ap(stg, (i % 2) * ZROW + 81,
                            [[3 * ZROW, 128], [112, H], [1, K]]),
                    in0=_ap(ews, (i % 2) * HK, [[2 * HK, 128], [K, H], [1, K]]),
                    in1=_ap(rzs, (i % 2) * H, [[2 * H, 128], [1, H], [0, K]]),
                    op=mybir.AluOpType.mult),
                waits=wts, inc=(s_dve, 1))

        def emit_zw(b, half):
            i = 2 * b + half

            def zw(e, b=b, half=half, i=i):
                return e.dma_start(
                    _ap(zdr, b * ZB6 + half * 128 * ZROW,
                        [[ZROW, 128], [1, ZROW]]),
                    _ap(stg, (i % 2) * ZROW, [[3 * ZROW, 128], [1, ZROW]]))
            m_zwm[(b, half)] = step("scalar", zw, waits=[m_nm[(b, half)]],
                                    inc=(s_zw2[b][half], 16))

        def emit_br(b, tc):
            idx = 4 * b + tc
            wts = [m_zwm[(b, 0)]]
            if tc > 0:
                wts.append(m_zwm[(b, 1)])
            if tc == 3:
                wts.append(m_pad)

            def br(e, b=b, tc=tc, idx=idx):
                src = _ap(zdr, b * ZB6 + 64 * tc * ZROW + 64 + WLO,
                          [[ZROW - 1, 64], [112, H], [1, WSUP]])
                dst = _ap(bnd, BD_W * idx,
                          [[16 * BD_W, 64], [WSUP, H], [1, WSUP]])
                return e.dma_start(dst, src)
            m_brm[idx] = step("sync", br, waits=wts, inc=(s_bri[idx], 16))

        def emit_trh(b):
            for j in range(3):
                wts = [m_act[(b, dc)] for dc in range(4)]
                wts += [m_id, slots.get(("pw", j))]

                def tr_h(e, b=b, j=j):
                    last = None
                    for dc in range(4):
                        in_ = _ap(hts, SLAB * dc + SLAB_T * b + 128 * j,
                                  [[4 * SLAB, 128], [1, JN[j]]])
                        last = nc.tensor.transpose(
                            pw_ap(j, (0, JN[j]), (128 * dc, 128 * dc + 128)),
                            in_, ident[:])
                    return last
                m = step("tensor", tr_h, waits=wts, inc=(s_pe, 1))
                slots[("trh", (b, j))] = m

            for tc in range(4):
                idx = 4 * b + tc
                lo = 64 * tc + WLO
                j0, r0 = divmod(lo, 128)
                take0 = min(WSUP, 128 - r0)
                wts = [slots[("trh", (b, j0))]]
                if take0 < WSUP:
                    wts.append(slots[("trh", (b, j0 + 1))])

                def cp_hbw(e, idx=idx, j0=j0, r0=r0, take0=take0):
                    last = nc.scalar.copy(
                        hbw[0:take0, 512 * idx:512 * idx + 512],
                        pw_ap(j0, (r0, r0 + take0), (0, 512)))
                    if take0 < WSUP:
                        last = nc.scalar.copy(
                            hbw[take0:WSUP, 512 * idx:512 * idx + 512],
                            pw_ap(j0 + 1, (0, WSUP - take0), (0, 512)))
                    return last
                m_cphbw[(b, tc)] = step("scalar", cp_hbw, waits=wts,
                                        inc=(s_act, 1))
            slots[("pw", 0)] = m_cphbw[(b, 1)]
            slots[("pw", 1)] = m_cphbw[(b, 3)]
            slots[("pw", 2)] = m_cphbw[(b, 3)]

        def emit_trb_cv(idx):
            b, tc = divmod(idx, 4)
            for half in range(2):
                wts = [m_brm[idx], m_id, slots.get(("pt", half))]

                def tr_b(e, idx=idx, half=half):
                    last = None
                    for hh in range(8):
                        h = 8 * half + hh
                        in_ = _ap(bnd, BD_W * idx + WSUP * h,
                                  [[16 * BD_W, 64], [1, WSUP]])
                        last = nc.tensor.transpose(
                            pt[half][:WSUP, 64 * hh:64 * hh + 64],
                            in_, ident[:64, :64])
                    return last
                m_tb = step("tensor", tr_b, waits=wts, inc=(s_pe, 1))
                m_cpbnd[(idx, half)] = step(
                    "vector",
                    lambda e, idx=idx, half=half: nc.vector.tensor_copy(
                        bndt[:WSUP, BDT_W * idx + 512 * half:
                             BDT_W * idx + 512 * half + 512],
                        pt[half][:WSUP, :512]),
                    waits=[m_tb], inc=(s_dve, 1))
                slots[("pt", half)] = m_cpbnd[(idx, half)]

            wts = [m_cpbnd[(idx, 0)], m_cpbnd[(idx, 1)],
                   m_cphbw[(b, tc)], slots.get(("pc", idx % 2))]

            def cv(e, idx=idx):
                last = None
                for h in range(H):
                    lhsT = hbw[0:WSUP, 512 * idx + 32 * h:512 * idx + 32 * h + 32]
                    rhs = bndt[0:WSUP, BDT_W * idx + 64 * h:BDT_W * idx + 64 * h + 64]
                    last = nc.tensor.matmul(
                        pc[idx % 2][32 * (h % 2):32 * (h % 2) + 32,
                                    64 * (h // 2):64 * (h // 2) + 64],
                        lhsT, rhs, start=True, stop=True)
                return last
            m_c = step("tensor", cv, waits=wts, inc=(s_pe, 1))

            def cvcp(e, idx=idx, b=b, tc=tc):
                # psum col-group cg holds heads (2cg, 2cg+1); cvt chunk
                # cg//2 at partition offset 64*(cg%2)
                last = None
                for cg in range(8):
                    cc = cg // 2
                    poff = 64 * (cg % 2)
                    last = nc.vector.tensor_copy(
                        _ap(cvt, poff * (4 * OWN) + OWN * cc + 256 * b + 64 * tc,
                            [[4 * OWN, 64], [1, TC]]),
                        pc[idx % 2][0:64, 64 * cg:64 * cg + 64])
                return last
            m_cvcp[idx] = step("vector", cvcp, waits=[m_c], inc=(s_dve, 1))
            slots[("pc", idx % 2)] = m_cvcp[idx]

        m_o2 = {}

        def emit_mm2(g2):
            for dc in range(4):
                pi = next_ph()
                wts = [m_cvcp[i] for i in range(8 * g2, 8 * g2 + 8)]
                wts += [m_w2, slots.get(("ph", pi))]

                def mm2(e, dc=dc, g2=g2, pi=pi):
                    last = None
                    for cc in range(4):
                        last = nc.tensor.matmul(
                            ph[pi][:, :512],
                            w2s[:, 512 * cc + 128 * dc:512 * cc + 128 * dc + 128],
                            _ap(cvt, OWN * cc + 512 * g2,
                                [[4 * OWN, 128], [1, 512]]),
                            start=(cc == 0), stop=(cc == 3))
                    return last
                m = step("tensor", mm2, waits=wts, inc=(s_pe, 1))
                m_o2[(g2, dc)] = step(
                    "scalar",
                    lambda e, dc=dc, g2=g2, pi=pi: nc.scalar.activation(
                        o2t[:, OWN * dc + 512 * g2:OWN * dc + 512 * g2 + 512],
                        ph[pi][:, :512],
                        mybir.ActivationFunctionType.Identity,
                        bias=b2s[:, dc:dc + 1], scale=1.0),
                    waits=[m, m_b2], inc=(s_act, 1))
                slots[("ph", pi)] = m_o2[(g2, dc)]

        def emit_store(g2):
            wts = [m_o2[(g2, dc)] for dc in range(4)]
            step("sync",
                 lambda e, g2=g2: e.dma_start(
                     _ap(out, 512 * g2, [[OWN, 128], [128 * OWN, 4], [1, 512]]),
                     _ap(o2t, 512 * g2, [[4 * OWN, 128], [OWN, 4], [1, 512]])),
                 waits=wts, inc=(s_out, 16))

        # ---------------- schedule ----------------
        for r in range(4):
            emit_mm1(r)
            emit_mmw(r, 0)
            emit_mmw(r, 1)
            emit_softmax(r, 0)
            emit_softmax(r, 1)
            emit_zw(r, 0)
            emit_zw(r, 1)
            for tc in range(4):
                emit_br(r, tc)
            emit_trh(r)
            if r >= 1:
                for tc in range(4):
                    emit_trb_cv(4 * (r - 1) + tc)
        emit_mm2(0)
        for tc in range(4):
            emit_trb_cv(12 + tc)
        emit_mm2(1)
        emit_store(0)
        emit_store(1)

        # ---------------- run per-engine step lists ----------------
        def runner(eng_obj, name):
            for emit, waits, inc in steps[name]:
                for sem, val in waits:
                    eng_obj.wait_ge(sem, val)
                inst = emit(eng_obj)
                if inc is not None:
                    sem, amt = inc
                    if inst is None:
                        inst = eng_obj.engine_nop()
                    inst.then_inc(sem, amt)

        @block.sync
        def _(eng):
            runner(eng, "sync")

        @block.vector
        def _(eng):
            runner(eng, "vector")

        @block.scalar
        def _(eng):
            runner(eng, "scalar")

        @block.gpsimd
        def _(eng):
            runner(eng, "gpsimd")

        @block.tensor
        def _(eng):
            runner(eng, "tensor")

    return nc


_NC = None


def _get_nc():
    global _NC
    if _NC is None:
        _NC = build_nc()
    return _NC


def _prep_inputs(x, W1, b1, Wl, W2, b2):
    xf = np.asarray(x, np.float32)
    xp = np.zeros((T + 2 * PAD, B, C), np.float32)
    xp[PAD:PAD + T] = xf
    w1t = np.ascontiguousarray(np.asarray(W1, np.float32).T).astype(ml_dtypes.bfloat16)
    wlt = np.ascontiguousarray(np.asarray(Wl, np.float32).T).astype(ml_dtypes.bfloat16)
    w2t = np.ascontiguousarray(np.asarray(W2, np.float32).T).astype(ml_dtypes.bfloat16)
    b1p = np.ascontiguousarray(np.asarray(b1, np.float32).reshape(4, 128).T)
    b2p = np.ascontiguousarray(np.asarray(b2, np.float32).reshape(4, 128).T)
    idp = np.eye(128, dtype=np.float32).astype(ml_dtypes.bfloat16)
    maps = []
    for c in range(NCORES):
        slab = xp[c * TLOC:c * TLOC + SLAB_T].transpose(1, 0, 2).reshape(SLAB, C)
        slabt = np.ascontiguousarray(slab.T).astype(ml_dtypes.bfloat16)
        maps.append({"xslabt": slabt, "w1t": w1t, "wlt": wlt, "w2t": w2t,
                     "b1p": b1p, "b2p": b2p, "idp": idp})
    return maps


def kernel(x, W1, b1, Wl, W2, b2):
    from concourse.bass_utils import run_bass_kernel_spmd
    nc = _get_nc()
    maps = _prep_inputs(x, W1, b1, Wl, W2, b2)
    res = run_bass_kernel_spmd(nc, maps, list(range(NCORES)))
    outs = [res.results[c]["out"].T.reshape(B, TLOC, C).transpose(1, 0, 2)
            for c in range(NCORES)]
    return np.concatenate(outs, axis=0)
